# revision 27
# baseline (speedup 1.0000x reference)
"""CAAN kernel for Trainium2, 8-core data-parallel (one batch row per core).

Math: the reference is
    Q = R Wq^T + bq ; K = R Wk^T + bk ; V = R Wv^T + bv
    E = exp(Q K^T / sqrt(512)) ; saat = E / rowsum(E)
    winner = (saat V) W1^T W2^T + (W2 b1 + b2)

Two algebraic collapses make most of the network disappear:

1. The W1/W2 head is linear, so with c = W1^T W2[0]:
       winner[n] = (sum_m E[n,m] u[m]) / (sum_m E[n,m]) + const,
   u = V c = R (Wv^T c) + bv.c — a per-asset scalar. The V projection and
   attention*V matmul vanish.

2. gamma = Q K^T = R A R^T + (R Wq^T bk)[n] + (R Wk^T bq)[m] + bq.bk with
   A = Wq^T Wk. The per-n term scales E rows uniformly and cancels in the
   s/rowsum ratio, so it is dropped. The per-m term v[m] rides the exp
   activation's per-partition bias slot. The Q and K projections collapse
   into a single projection B = A^T-pack @ R^T.

fp8 version: everything the PE touches is fp8-e4m3 (A and the u/v weights
pre-scaled by 64 to clear the e4m3 subnormal range). All big matmuls run
in DoubleRow perf mode (256 contraction rows per matmul) for 2x PE
throughput; the s/rowsum reduction is DoubleRow too (exp output ET is
fp8, two m-chunks per matmul). rel-err vs the f32 reference ~4.4e-3: fp8
quantization noise averages out in the softmax sums.

The critical engine end-to-end is ACT: 32 exp ops of [128,1024] (two PSUM
banks per op, amortizing the ~300ns per-op overhead) = ~37us that nothing
can shrink, so the whole schedule exists to (a) start the first exp as
early as possible and (b) never let ACT starve. Per core:
  warmup: 14 dummy matmuls un-throttle the PE HAM clock gate during the
          input-DMA ramp (cold PE runs at half clock for ~3.4us).
  uv:     8 DoubleRow matmuls -> RAW rows 64*R.wtilde / 64*R.w2tilde in
          PSUM rows 0/32. No on-device affine fixup: the 1/64 and +beta on
          u ride to the host via s = s' + beta*rowsum, v's scale folds into
          the e^v activation, and e^{SCALE*beta2} cancels in s/rowsum.
          Rows bounce through DRAM to become [128, pair, sub] fp8 columns;
          e^v then folds INTO the s/rowsum weights (su col0 = u_raw*e^v,
          col1 = e^v), keeping the v chain off the exp critical path.
  proj:   B = A R^T, DoubleRow. First n-half (m 0:1024) in psMain with
          casts split DVE/ACT per 512 so gamma(0) unblocks ~1us after the
          wave; second n-half runs as fillers through the idle psR banks so
          its DVE casts never gate the gamma psMain rotation.
  gamma:  per m-chunk: 8 DoubleRow matmuls into two [128,1024] 2-bank
          psMain tiles; exp(psum * SCALE/64) -> fp8 ET, 2-bank ACT ops.
          The two psMain tiles rotate at exactly ACT speed.
  srs:    s (row 0) and rowsum (row 1) accumulate over 8 chunk-pairs of
          DoubleRow matmuls with [u*e^v|e^v|0...] weights, spread through
          the gamma stream as PE fillers trailing their exps.
  out:    [2, 2048] f32 DMA'd out; host does
          winner = s/(64*rowsum) + beta + const.
"""

import math

import ml_dtypes
import numpy as np

import concourse.bass as bass
import concourse.mybir as mybir
import concourse.tile as tile
from concourse.bass_utils import run_bass_kernel_spmd
from concourse.vector_clock import ScopedClock


N_CORES = 8
NB, NN, DD = 8, 2048, 512  # batch, assets, feature dim
P = 128
NQ = DD // P   # q chunks (contraction)
NM = NN // P   # m chunks (key/asset rows)
NPR = NM // 2  # m chunk-pairs for the DoubleRow s/rowsum reduction
S = 512        # PSUM bank width in f32
NS = NN // S   # n slices of 512
WUVC = 48      # wuv padded col count (16-aligned for DoubleRow weight step)
SUC = 16       # su padded col count (16-aligned DoubleRow weight step)
BF16 = mybir.dt.bfloat16
F32 = mybir.dt.float32
F8 = mybir.dt.float8e4
SCALE = 1.0 / math.sqrt(float(DD))
ASCALE = 64.0  # fp8 pre-scale on A / wuv (keeps entries out of e4m3 subnormals)
BF = ml_dtypes.bfloat16
F8NP = ml_dtypes.float8_e4m3
DR = mybir.MatmulPerfMode.DoubleRow


class _TileContext(tile.TileContext):
    """Workaround for walrus rejecting >1 sem wait on the kernel-tail Drain
    ("Too many sync wait commands"): put each final wait on its own SP NoOp
    ahead of an unwaited Drain."""

    def _drain_and_barrier(self, tick_clock, wait_clock):
        nc = self.nc
        probe = nc.sync.nop(nofuse=True)
        wait_clock.add_sem_waits(
            probe.ins, ScopedClock({None: tick_clock.global_clock})
        )
        si = probe.ins.sync_info
        waits = list(si.on_wait) if si is not None else []
        if si is not None:
            si.on_wait = []
        # spread the final waits round-robin over all engines so they
        # resolve in parallel; the barrier then guarantees every wait has
        # been observed before the SP drain runs.
        engines = [nc.sync, nc.vector, nc.scalar, nc.tensor, nc.gpsimd]
        for i, w in enumerate(waits):
            n = engines[i % len(engines)].nop(nofuse=True)
            n.ins.sync_info = mybir.SyncInfo(on_wait=[w], on_update=[])
        nc.all_engine_barrier()
        nc.sync.drain()
        assert self.sems is not None
        popped = nc._tile_sem_poison_stack.pop()
        assert popped is self._sem_poison
        # clear_and_free_semaphores would range-clear every ALLOCATED sem id
        # (~200+), which walrus lowers to one op per id (~7us of tail).
        # Only ids that appear in the final instruction stream can be
        # non-zero, so hardware-clear just those; do the allocator
        # bookkeeping for the full set.
        allocated = list(self.sems.allocated().values())
        sem_nums = [
            s.num if hasattr(s, "num") else int(s) for s in allocated
        ]
        used = set()
        for fn in nc.m.functions:
            for blk in fn.blocks:
                for inst in blk.instructions:
                    si = inst.sync_info
                    if si is not None:
                        for w in si.on_wait:
                            used.add(w.id)
                        for u in si.on_update:
                            used.add(u.id)
        hw_nums = sorted(n for n in sem_nums if n in used)
        for sem_range in bass.compact_to_ranges(hw_nums):
            nc.gpsimd.dma_reset(sem_range)
            nc.gpsimd.sem_clear(sem_range)
        nc._state.prepend_free_semaphores(sem_nums)
        for poison_set in nc._tile_sem_poison_stack:
            poison_set.update(sem_nums)
        # the trailing all_engine_barrier is skipped: nothing after the
        # clear touches semaphores, and the runtime serializes executions


def _split_multi_waits(nc, maxw=1):
    """This walrus build rejects instructions carrying more than one sync
    wait ("Too many sync wait commands"). Move excess waits onto same-engine
    NoOps inserted just before the instruction: sem-ge waits are monotonic
    within the kernel, so waiting for them earlier on the same engine is
    equivalent. sem-eq waits stay on the original instruction."""
    for fn in nc.m.functions:
        for blk in fn.blocks:
            insts = blk.instructions
            if not any(
                i.sync_info is not None and len(i.sync_info.on_wait) > maxw
                for i in insts
            ):
                continue
            out = []
            for inst in insts:
                si = inst.sync_info
                if si is not None and len(si.on_wait) > maxw:
                    keep = [w for w in si.on_wait if "eq" in w.wait_mode]
                    movable = [w for w in si.on_wait if "eq" not in w.wait_mode]
                    while len(keep) < maxw and movable:
                        keep.append(movable.pop(0))
                    assert len(keep) <= maxw, (
                        f"{inst.name}: {len(keep)} non-splittable waits"
                    )
                    for w in movable:
                        nop = mybir.InstNoOp(
                            name=nc.get_next_instruction_name(), ins=[], outs=[]
                        )
                        nop.engine = inst.engine
                        nop.sync_info = mybir.SyncInfo(on_wait=[w], on_update=[])
                        out.append(nop)
                    si.on_wait = keep
                out.append(inst)
            blk.instructions = out


def _build():
    nc = bass.Bass("TRN2", target_bir_lowering=False, debug=False)

    rt = nc.dram_tensor("rt", (P, NS, NQ, S), F8, kind="ExternalInput")
    amat = nc.dram_tensor("amat", (P, NQ, DD), F8, kind="ExternalInput")
    wuv = nc.dram_tensor("wuv", (P, NQ, WUVC), F8, kind="ExternalInput")
    out = nc.dram_tensor("out", (2, NN), F32, kind="ExternalOutput")

    Ident = mybir.ActivationFunctionType.Identity
    Exp = mybir.ActivationFunctionType.Exp

    with _TileContext(nc) as tc:
        with (
            tc.tile_pool(name="const", bufs=1) as cpool,
            tc.tile_pool(name="big", bufs=1) as big,
            tc.tile_pool(name="et", bufs=6) as et_pool,
            tc.tile_pool(name="dscratch", bufs=1, space="DRAM") as dpool,
        ):
            rt_sb = cpool.tile([P, NS, NQ, S], F8, name="rt")
            a_sb = cpool.tile([P, NQ, DD], F8, name="a")
            wuv_sb = cpool.tile([P, NQ, WUVC], F8, name="wuv")
            dummy_sb = cpool.tile([P, S], BF16, name="dummy")

            # DMA order matches consumption order: wuv + the first rt
            # pair-slices feed the uv matmuls, a feeds proj wave 0.
            # rt is packed [p, n-slice, chunk, 512] so each n-slice is one
            # 256KB DMA with 2KB-contiguous partition lines (512B lines ran
            # at ~1/4 the bandwidth and starved the uv/proj ramp). `a` goes
            # first on its queue: it gates the entire proj->gamma->exp
            # pipeline, while wuv/betas feed later consumers. betas (33 tiny
            # 8B packets, slow to drain) rides behind the rt slices on sync.
            # slices split pair-wise across all three queues (contiguous
            # 1KB partition lines), laid out so ns0 AND ns1 land almost
            # together (~10.5us): they gate uv01+np0 and hence the whole
            # proj->gamma->exp chain.
            nc.scalar.dma_start(wuv_sb[:], wuv.ap())
            nc.sync.dma_start(rt_sb[:, 0, 0:2], rt.ap()[:, 0, 0:2])
            nc.gpsimd.dma_start(rt_sb[:, 0, 2:4], rt.ap()[:, 0, 2:4])
            nc.scalar.dma_start(rt_sb[:, 1, 2:4], rt.ap()[:, 1, 2:4])
            nc.gpsimd.dma_start(rt_sb[:, 1, 0:2], rt.ap()[:, 1, 0:2])
            nc.scalar.dma_start(a_sb[:], amat.ap())
            nc.sync.dma_start(rt_sb[:, 2, 0:2], rt.ap()[:, 2, 0:2])
            nc.gpsimd.dma_start(rt_sb[:, 2, 2:4], rt.ap()[:, 2, 2:4])
            nc.sync.dma_start(rt_sb[:, 3, 0:2], rt.ap()[:, 3, 0:2])
            nc.sync.dma_start(rt_sb[:, 3, 2:4], rt.ap()[:, 3, 2:4])

            bt_sb = big.tile([P, NQ, NN], F8, name="bt")
            uvrow_sb = big.tile([33, NN], F8, name="uvrow")
            vcol_sb = big.tile([P, NPR, 2], F8, name="vcol")
            ev_sb = big.tile([P, NPR, 2], F8, name="ev")
            # su weights per m-chunk-pair: [pair, sub, col] with col 0 =
            # u*e^v, col 1 = e^v, cols 2..15 zero padding (16B DoubleRow
            # step). Folding e^v into the weights instead of biasing the exp
            # keeps the v scatter chain off the first exp's critical path:
            #   E[n,m] = e^{v[m]} E'[n,m]  =>  s = E'.(e^v u), rowsum = E'.e^v
            su_sb = big.tile([P, NPR, 2, SUC], F8, name="su")
            nc.gpsimd.memset(su_sb[:], 0.0)

            # psR: 4 banks for the s/rowsum accumulators (also reused as
            # scratch for the uv projections before srs starts).
            # psMain: 4 banks as two rotating [128,1024] 2-bank tiles so the
            # exp ACTs can cover 1024 columns per instruction.
            psR = tc.alloc_tile_pool(name="psR", bufs=1, space="PSUM")
            psMain = tc.alloc_tile_pool(name="psMain", bufs=2, space="PSUM")
            srs = [
                psR.tile([P, S], F32, tag=f"srs{ns}", name=f"srs{ns}")
                for ns in range(NS)
            ]

            # ---- PE warmup: dummy matmuls (no input deps) un-throttle the
            # HAM clock gate while the input DMAs stream in.
            nc.vector.memset(dummy_sb[:], 0.0)
            for _ in range(8):
                t = psMain.tile([P, 2 * S], F32, tag="mm", name="mm")
                nc.tensor.matmul(
                    t[:, 0:S], dummy_sb[:, 0:P], dummy_sb[:], start=True, stop=True
                )

            # ---- uv: raw u row (64*R.wtilde, row 0) and raw v row
            # (64*R.w2tilde, row 32). No on-device affine fixup at all: the
            # 1/64 and +beta on u ride to the host (s = s' + beta*rowsum),
            # and v's scale folds into the e^v activation while e^{beta2}
            # cancels in the s/rowsum ratio.
            def uv(ns):
                pur = srs[ns][0:WUVC, :]
                for pr in range(2):
                    nc.tensor.matmul(
                        pur[:],
                        wuv_sb[:, 2 * pr : 2 * pr + 2, :],
                        rt_sb[:, ns, 2 * pr : 2 * pr + 2, :],
                        start=(pr == 0),
                        stop=(pr == 1),
                        perf_mode=DR,
                        skip_group_check=True,
                    )

            def uv_copy(ns):
                nc.vector.tensor_copy(
                    uvrow_sb[0:33, ns * S : (ns + 1) * S], srs[ns][0:33, :]
                )

            def uv_scatter():
                # scatter rows [1, 2048] -> [128, pair, sub] columns off the
                # PE: bounce through flat DRAM, where the partition-scatter
                # read pattern is expressible. The two 2048-descriptor
                # scatters go on different queues so they don't serialize;
                # results are needed only by the first srs matmul, ~10us out.
                uv_dram = dpool.tile([2, NN], F8)
                nc.sync.dma_start(uv_dram[1:2, :], uvrow_sb[32:33, :])
                nc.sync.dma_start(uv_dram[0:1, :], uvrow_sb[0:1, :])
                with nc.allow_non_contiguous_dma(
                    reason="2048-elem partition scatter, one-off"
                ):
                    nc.sync.dma_start(
                        vcol_sb[:, :, :],
                        uv_dram[1, :].rearrange("(pr i p) -> p pr i", p=P, i=2),
                    )
                    nc.gpsimd.dma_start(
                        su_sb[:, :, :, 0],
                        uv_dram[0, :].rearrange("(pr i p) -> p pr i", p=P, i=2),
                    )

            def su_fold():
                # ev = e^v, then su col0 = u*e^v, col1 = e^v. One tiny ACT
                # (between exps on the scalar queue) + two DVE ops.
                nc.scalar.activation(
                    ev_sb[:, :, :],
                    vcol_sb[:, :, :],
                    Exp,
                    bias=0.0,
                    scale=SCALE / ASCALE,
                )
                nc.vector.tensor_tensor(
                    su_sb[:, :, :, 0],
                    su_sb[:, :, :, 0],
                    ev_sb[:, :, :],
                    mybir.AluOpType.mult,
                )
                nc.vector.tensor_copy(su_sb[:, :, :, 1], ev_sb[:, :, :])

            # ---- proj: B = A R^T, DoubleRow, cast to fp8 bt ----
            def proj(qo, half, cast_on_act=False):
                t = psMain.tile([P, 2 * S], F32, tag="mm", name="mm")
                for sub in range(2):
                    ns = 2 * half + sub
                    po = t[:, sub * S : (sub + 1) * S]
                    for pr in range(2):
                        nc.tensor.matmul(
                            po[:],
                            a_sb[:, 2 * pr : 2 * pr + 2, qo * P : (qo + 1) * P],
                            rt_sb[:, ns, 2 * pr : 2 * pr + 2, :],
                            start=(pr == 0),
                            stop=(pr == 1),
                            perf_mode=DR,
                        )
                # split each cast across DVE and the (still idle) ACT engine:
                # gamma chunks 0-3 need only the first 512 columns, so the
                # DVE halves unblock the first gamma chunk ~1us after the
                # wave's last matmul
                lo = bt_sb[:, qo, half * 2 * S : half * 2 * S + S]
                hi = bt_sb[:, qo, half * 2 * S + S : (half + 1) * 2 * S]
                nc.vector.tensor_copy(lo, t[:, 0:S])
                nc.scalar.copy(hi, t[:, S : 2 * S])

            def proj_psr(qo, half, banks):
                # second-half proj waves write the (still idle) psR banks so
                # their DVE casts never gate the gamma psMain rotation
                for sub in range(2):
                    ns = 2 * half + sub
                    po = srs[banks[sub]]
                    for pr in range(2):
                        nc.tensor.matmul(
                            po[:],
                            a_sb[:, 2 * pr : 2 * pr + 2, qo * P : (qo + 1) * P],
                            rt_sb[:, ns, 2 * pr : 2 * pr + 2, :],
                            start=(pr == 0),
                            stop=(pr == 1),
                            perf_mode=DR,
                            skip_group_check=True,
                        )
                    nc.vector.tensor_copy(
                        bt_sb[:, qo, ns * S : (ns + 1) * S], po[:]
                    )

            # ---- gamma + exp ----
            ets = {}

            def gamma(mc):
                if mc % 2 == 0:
                    ets[mc // 2] = et_pool.tile(
                        [P, 2, NN], F8, tag="et", name="et"
                    )
                et = ets[mc // 2]
                for half in range(2):
                    g = psMain.tile([P, 2 * S], F32, tag="mm", name="mm")
                    for sub in range(2):
                        ns = 2 * half + sub
                        go = g[:, sub * S : (sub + 1) * S]
                        for pr in range(2):
                            nc.tensor.matmul(
                                go[:],
                                bt_sb[:, 2 * pr : 2 * pr + 2, mc * P : (mc + 1) * P],
                                rt_sb[:, ns, 2 * pr : 2 * pr + 2, :],
                                start=(pr == 0),
                                stop=(pr == 1),
                                perf_mode=DR,
                            )
                    nc.scalar.activation(
                        et[:, mc % 2, half * 2 * S : (half + 1) * 2 * S],
                        g[:],
                        Exp,
                        bias=0.0,
                        scale=SCALE / ASCALE,
                    )

            # ---- srs: s/rowsum accumulation over m chunk-pairs ----
            def srs_pair(pp):
                et = ets[pp]
                for ns in range(NS):
                    nc.tensor.matmul(
                        srs[ns][0:SUC, :],
                        su_sb[:, pp, :, :],
                        et[:, :, ns * S : (ns + 1) * S],
                        start=(pp == 0),
                        stop=(pp == NPR - 1),
                        perf_mode=DR,
                        skip_group_check=True,
                    )
                del ets[pp]

            # schedule, ordered to (a) match DMA arrival, (b) start the exp
            # stream (the critical ACT work) as early as possible, and (c)
            # spread the remaining proj/srs PE work thinly through the gamma
            # stream so ACT never starves: per gamma chunk the PE owes 1.73us
            # against ACT's 2.3us of exp, so every other chunk gets one 4-MM
            # filler (a proj wave-half or an srs pair).
            uv(0)
            uv(1)
            for qo in range(NQ):
                proj(qo, 0, cast_on_act=(qo % 2 == 1))
            gamma(0)
            uv(2)
            uv(3)
            for ns in range(NS):
                uv_copy(ns)
            uv_scatter()
            proj_psr(0, 1, (0, 1))
            gamma(1)
            proj_psr(1, 1, (2, 3))
            gamma(2)
            proj_psr(2, 1, (0, 1))
            gamma(3)
            proj_psr(3, 1, (2, 3))
            gamma(4)
            su_fold()
            # srs pair p must trail gamma(2p+3) so its exps are done when the
            # in-order PE queue reaches it
            srs_slot = {7: 0, 8: 1, 9: 2, 10: 3, 11: 4, 13: 5, 15: 6}
            for mc in range(5, 16):
                gamma(mc)
                if mc in srs_slot:
                    srs_pair(srs_slot[mc])
            srs_pair(NPR - 1)

            # copy the two result rows PSUM -> SBUF, then one DMA out.
            out_sb = big.tile([2, NN], F32, name="out_sb")
            for ns in range(NS):
                sl = slice(ns * S, (ns + 1) * S)
                # alternate DVE/ACT so the four drain copies run on two
                # engines in parallel
                if ns % 2 == 0:
                    nc.vector.tensor_copy(out_sb[:, sl], srs[ns][0:2, :])
                else:
                    nc.scalar.copy(out_sb[:, sl], srs[ns][0:2, :])
            nc.sync.dma_start(out.ap()[:], out_sb[:])
            psMain.release()
            psR.release()

    _split_multi_waits(nc)
    return nc


_NC = None


def _get_nc():
    global _NC
    if _NC is None:
        _NC = _build()
    return _NC


def kernel(R, Wq, bq, Wk, bk, Wv, bv, W1, b1, W2, b2):
    R = np.asarray(R, np.float32)
    Wq = np.asarray(Wq, np.float64)
    bq = np.asarray(bq, np.float64)
    Wk = np.asarray(Wk, np.float64)
    bk = np.asarray(bk, np.float64)
    Wv = np.asarray(Wv, np.float64)
    bv = np.asarray(bv, np.float64)
    W1 = np.asarray(W1, np.float64)
    b1 = np.asarray(b1, np.float64)
    W2 = np.asarray(W2, np.float64)
    b2 = np.asarray(b2, np.float64)

    # Collapse the linear head: winner = c.a + const, u = V c.
    c = W1.T @ W2[0]                      # [512]
    wtilde = Wv.T @ c                     # [512]
    beta = float(bv @ c)
    const = float(W2[0] @ b1 + b2[0])
    # Collapse the Q/K projections: gamma = R A R^T + v[m] (+ dropped n-term)
    at = Wk.T @ Wq                        # A^T = Wk^T Wq, [q', q]
    w2tilde = Wk.T @ bq                   # [512]
    beta2 = float(bq @ bk)

    # [p, chunk, d] with q = chunk*128 + p; x64 pre-scale for fp8 range
    a_h = np.ascontiguousarray(
        (at * ASCALE).reshape(NQ, P, DD).transpose(1, 0, 2)
    ).astype(F8NP)
    wuv_h = np.zeros((P, NQ, WUVC), F8NP)
    wuv_h[:, :, 0] = (wtilde * ASCALE).reshape(NQ, P).T.astype(F8NP)
    wuv_h[:, :, 32] = (w2tilde * ASCALE).reshape(NQ, P).T.astype(F8NP)

    in_maps = []
    for b in range(NB):
        # [p, chunk, n]: R[b].T chunked over q so each q-chunk-pair slice
        # is a regular strided DMA
        rt_h = np.ascontiguousarray(
            R[b].T.reshape(NQ, P, NS, S).transpose(1, 2, 0, 3)
        ).astype(F8NP)
        in_maps.append(
            {
                "rt": rt_h,
                "amat": a_h,
                "wuv": wuv_h,
            }
        )

    nc = _get_nc()
    res = run_bass_kernel_spmd(nc, in_maps, core_ids=list(range(N_CORES)))
    outs = np.stack([res.results[b]["out"] for b in range(NB)])   # [8,2,2048]
    # s row is 64x (raw u weights); beta rides here via s = s' + beta*rowsum
    return (
        outs[:, 0] / (np.float32(ASCALE) * outs[:, 1])
        + np.float32(beta + const)
    ).astype(np.float32)


# revision 28
# speedup vs baseline: 1.0342x; 1.0342x over previous
"""CAAN kernel for Trainium2, 8-core data-parallel (one batch row per core).

Math: the reference is
    Q = R Wq^T + bq ; K = R Wk^T + bk ; V = R Wv^T + bv
    E = exp(Q K^T / sqrt(512)) ; saat = E / rowsum(E)
    winner = (saat V) W1^T W2^T + (W2 b1 + b2)

Two algebraic collapses make most of the network disappear:

1. The W1/W2 head is linear, so with c = W1^T W2[0]:
       winner[n] = (sum_m E[n,m] u[m]) / (sum_m E[n,m]) + const,
   u = V c = R (Wv^T c) + bv.c — a per-asset scalar. The V projection and
   attention*V matmul vanish.

2. gamma = Q K^T = R A R^T + (R Wq^T bk)[n] + (R Wk^T bq)[m] + bq.bk with
   A = Wq^T Wk. The per-n term scales E rows uniformly and cancels in the
   s/rowsum ratio, so it is dropped. The per-m term v[m] rides the exp
   activation's per-partition bias slot. The Q and K projections collapse
   into a single projection B = A^T-pack @ R^T.

fp8 version: everything the PE touches is fp8-e4m3 (A and the u/v weights
pre-scaled by 64 to clear the e4m3 subnormal range). All big matmuls run
in DoubleRow perf mode (256 contraction rows per matmul) for 2x PE
throughput; the s/rowsum reduction is DoubleRow too (exp output ET is
fp8, two m-chunks per matmul). rel-err vs the f32 reference ~4.4e-3: fp8
quantization noise averages out in the softmax sums.

The critical engine end-to-end is ACT: 32 exp ops of [128,1024] (two PSUM
banks per op, amortizing the ~300ns per-op overhead) = ~37us that nothing
can shrink, so the whole schedule exists to (a) start the first exp as
early as possible and (b) never let ACT starve. Per core:
  warmup: 14 dummy matmuls un-throttle the PE HAM clock gate during the
          input-DMA ramp (cold PE runs at half clock for ~3.4us).
  uv:     8 DoubleRow matmuls -> RAW rows 64*R.wtilde / 64*R.w2tilde in
          PSUM rows 0/32. No on-device affine fixup: the 1/64 and +beta on
          u ride to the host via s = s' + beta*rowsum, v's scale folds into
          the e^v activation, and e^{SCALE*beta2} cancels in s/rowsum.
          Rows bounce through DRAM to become [128, pair, sub] fp8 columns;
          e^v then folds INTO the s/rowsum weights (su col0 = u_raw*e^v,
          col1 = e^v), keeping the v chain off the exp critical path.
  proj:   B = A R^T, DoubleRow. First n-half (m 0:1024) in psMain with
          casts split DVE/ACT per 512 so gamma(0) unblocks ~1us after the
          wave; second n-half runs as fillers through the idle psR banks so
          its DVE casts never gate the gamma psMain rotation.
  gamma:  per m-chunk: 8 DoubleRow matmuls into two [128,1024] 2-bank
          psMain tiles; exp(psum * SCALE/64) -> fp8 ET, 2-bank ACT ops.
          The two psMain tiles rotate at exactly ACT speed.
  srs:    s (row 0) and rowsum (row 1) accumulate over 8 chunk-pairs of
          DoubleRow matmuls with [u*e^v|e^v|0...] weights, spread through
          the gamma stream as PE fillers trailing their exps.
  out:    [2, 2048] f32 DMA'd out; host does
          winner = s/(64*rowsum) + beta + const.
"""

import math

import ml_dtypes
import numpy as np

import concourse.bass as bass
import concourse.mybir as mybir
import concourse.tile as tile
from concourse.bass_utils import run_bass_kernel_spmd
from concourse.vector_clock import ScopedClock


N_CORES = 8
NB, NN, DD = 8, 2048, 512  # batch, assets, feature dim
P = 128
NQ = DD // P   # q chunks (contraction)
NM = NN // P   # m chunks (key/asset rows)
NPR = NM // 2  # m chunk-pairs for the DoubleRow s/rowsum reduction
S = 512        # PSUM bank width in f32
NS = NN // S   # n slices of 512
WUVC = 48      # wuv padded col count (16-aligned for DoubleRow weight step)
SUC = 16       # su padded col count (16-aligned DoubleRow weight step)
BF16 = mybir.dt.bfloat16
F32 = mybir.dt.float32
F8 = mybir.dt.float8e4
SCALE = 1.0 / math.sqrt(float(DD))
ASCALE = 64.0  # fp8 pre-scale on A / wuv (keeps entries out of e4m3 subnormals)
BF = ml_dtypes.bfloat16
F8NP = ml_dtypes.float8_e4m3
DR = mybir.MatmulPerfMode.DoubleRow


class _TileContext(tile.TileContext):
    """Workaround for walrus rejecting >1 sem wait on the kernel-tail Drain
    ("Too many sync wait commands"): put each final wait on its own SP NoOp
    ahead of an unwaited Drain."""

    def _drain_and_barrier(self, tick_clock, wait_clock):
        nc = self.nc
        probe = nc.sync.nop(nofuse=True)
        wait_clock.add_sem_waits(
            probe.ins, ScopedClock({None: tick_clock.global_clock})
        )
        si = probe.ins.sync_info
        waits = list(si.on_wait) if si is not None else []
        if si is not None:
            si.on_wait = []
        # spread the final waits round-robin over all engines so they
        # resolve in parallel; the barrier then guarantees every wait has
        # been observed before the SP drain runs.
        engines = [nc.sync, nc.vector, nc.scalar, nc.tensor, nc.gpsimd]
        for i, w in enumerate(waits):
            n = engines[i % len(engines)].nop(nofuse=True)
            n.ins.sync_info = mybir.SyncInfo(on_wait=[w], on_update=[])
        nc.all_engine_barrier()
        nc.sync.drain()
        assert self.sems is not None
        popped = nc._tile_sem_poison_stack.pop()
        assert popped is self._sem_poison
        # clear_and_free_semaphores would range-clear every ALLOCATED sem id
        # (~200+), which walrus lowers to one op per id (~7us of tail).
        # Only ids that appear in the final instruction stream can be
        # non-zero, so hardware-clear just those; do the allocator
        # bookkeeping for the full set.
        allocated = list(self.sems.allocated().values())
        sem_nums = [
            s.num if hasattr(s, "num") else int(s) for s in allocated
        ]
        used = set()
        for fn in nc.m.functions:
            for blk in fn.blocks:
                for inst in blk.instructions:
                    si = inst.sync_info
                    if si is not None:
                        for w in si.on_wait:
                            used.add(w.id)
                        for u in si.on_update:
                            used.add(u.id)
        hw_nums = sorted(n for n in sem_nums if n in used)
        for sem_range in bass.compact_to_ranges(hw_nums):
            nc.gpsimd.dma_reset(sem_range)
            nc.gpsimd.sem_clear(sem_range)
        nc._state.prepend_free_semaphores(sem_nums)
        for poison_set in nc._tile_sem_poison_stack:
            poison_set.update(sem_nums)
        # the trailing all_engine_barrier is skipped: nothing after the
        # clear touches semaphores, and the runtime serializes executions


def _split_multi_waits(nc, maxw=1):
    """This walrus build rejects instructions carrying more than one sync
    wait ("Too many sync wait commands"). Move excess waits onto same-engine
    NoOps inserted just before the instruction: sem-ge waits are monotonic
    within the kernel, so waiting for them earlier on the same engine is
    equivalent. sem-eq waits stay on the original instruction."""
    for fn in nc.m.functions:
        for blk in fn.blocks:
            insts = blk.instructions
            if not any(
                i.sync_info is not None and len(i.sync_info.on_wait) > maxw
                for i in insts
            ):
                continue
            out = []
            for inst in insts:
                si = inst.sync_info
                if si is not None and len(si.on_wait) > maxw:
                    keep = [w for w in si.on_wait if "eq" in w.wait_mode]
                    movable = [w for w in si.on_wait if "eq" not in w.wait_mode]
                    while len(keep) < maxw and movable:
                        keep.append(movable.pop(0))
                    assert len(keep) <= maxw, (
                        f"{inst.name}: {len(keep)} non-splittable waits"
                    )
                    for w in movable:
                        nop = mybir.InstNoOp(
                            name=nc.get_next_instruction_name(), ins=[], outs=[]
                        )
                        nop.engine = inst.engine
                        nop.sync_info = mybir.SyncInfo(on_wait=[w], on_update=[])
                        out.append(nop)
                    si.on_wait = keep
                out.append(inst)
            blk.instructions = out


def _build():
    nc = bass.Bass("TRN2", target_bir_lowering=False, debug=False)

    rt = nc.dram_tensor("rt", (P, NS, NQ, S), F8, kind="ExternalInput")
    amat = nc.dram_tensor("amat", (P, NQ, DD), F8, kind="ExternalInput")
    wuv = nc.dram_tensor("wuv", (P, NQ, WUVC), F8, kind="ExternalInput")
    out = nc.dram_tensor("out", (2, NN), F32, kind="ExternalOutput")

    Ident = mybir.ActivationFunctionType.Identity
    Exp = mybir.ActivationFunctionType.Exp

    with _TileContext(nc) as tc:
        with (
            tc.tile_pool(name="const", bufs=1) as cpool,
            tc.tile_pool(name="big", bufs=1) as big,
            tc.tile_pool(name="et", bufs=6) as et_pool,
            tc.tile_pool(name="dscratch", bufs=1, space="DRAM") as dpool,
        ):
            rt_sb = cpool.tile([P, NS, NQ, S], F8, name="rt")
            a_sb = cpool.tile([P, NQ, DD], F8, name="a")
            wuv_sb = cpool.tile([P, NQ, WUVC], F8, name="wuv")
            dummy_sb = cpool.tile([P, S], BF16, name="dummy")

            # DMA order matches consumption order: wuv + the first rt
            # pair-slices feed the uv matmuls, a feeds proj wave 0.
            # rt is packed [p, n-slice, chunk, 512] so each n-slice is one
            # 256KB DMA with 2KB-contiguous partition lines (512B lines ran
            # at ~1/4 the bandwidth and starved the uv/proj ramp). `a` goes
            # first on its queue: it gates the entire proj->gamma->exp
            # pipeline, while wuv/betas feed later consumers. betas (33 tiny
            # 8B packets, slow to drain) rides behind the rt slices on sync.
            # each n-slice splits pair-wise across both queues (contiguous
            # 1KB partition lines) so slices arrive in consumption order at
            # the aggregate bandwidth of both queues
            nc.scalar.dma_start(wuv_sb[:], wuv.ap())
            nc.scalar.dma_start(a_sb[:], amat.ap())
            for ns in range(NS):
                e0, e1 = (nc.sync, nc.gpsimd) if ns % 2 == 0 else (nc.gpsimd, nc.sync)
                e0.dma_start(rt_sb[:, ns, 0:2], rt.ap()[:, ns, 0:2])
                e1.dma_start(rt_sb[:, ns, 2:4], rt.ap()[:, ns, 2:4])

            bt_sb = big.tile([P, NQ, NN], F8, name="bt")
            uvrow_sb = big.tile([33, NN], F8, name="uvrow")
            vcol_sb = big.tile([P, NPR, 2], F8, name="vcol")
            ev_sb = big.tile([P, NPR, 2], F8, name="ev")
            # su weights per m-chunk-pair: [pair, sub, col] with col 0 =
            # u*e^v, col 1 = e^v, cols 2..15 zero padding (16B DoubleRow
            # step). Folding e^v into the weights instead of biasing the exp
            # keeps the v scatter chain off the first exp's critical path:
            #   E[n,m] = e^{v[m]} E'[n,m]  =>  s = E'.(e^v u), rowsum = E'.e^v
            su_sb = big.tile([P, NPR, 2, SUC], F8, name="su")
            nc.gpsimd.memset(su_sb[:], 0.0)

            # psR: 4 banks for the s/rowsum accumulators (also reused as
            # scratch for the uv projections before srs starts).
            # psMain: 4 banks as two rotating [128,1024] 2-bank tiles so the
            # exp ACTs can cover 1024 columns per instruction.
            psR = tc.alloc_tile_pool(name="psR", bufs=1, space="PSUM")
            psMain = tc.alloc_tile_pool(name="psMain", bufs=2, space="PSUM")
            srs = [
                psR.tile([P, S], F32, tag=f"srs{ns}", name=f"srs{ns}")
                for ns in range(NS)
            ]

            # ---- PE warmup: dummy matmuls (no input deps) un-throttle the
            # HAM clock gate while the input DMAs stream in.
            nc.vector.memset(dummy_sb[:], 0.0)
            for _ in range(14):
                t = psMain.tile([P, 2 * S], F32, tag="mm", name="mm")
                nc.tensor.matmul(
                    t[:, 0:S], dummy_sb[:, 0:P], dummy_sb[:], start=True, stop=True
                )

            # ---- uv: raw u row (64*R.wtilde, row 0) and raw v row
            # (64*R.w2tilde, row 32). No on-device affine fixup at all: the
            # 1/64 and +beta on u ride to the host (s = s' + beta*rowsum),
            # and v's scale folds into the e^v activation while e^{beta2}
            # cancels in the s/rowsum ratio.
            def uv(ns):
                pur = srs[ns][0:WUVC, :]
                for pr in range(2):
                    nc.tensor.matmul(
                        pur[:],
                        wuv_sb[:, 2 * pr : 2 * pr + 2, :],
                        rt_sb[:, ns, 2 * pr : 2 * pr + 2, :],
                        start=(pr == 0),
                        stop=(pr == 1),
                        perf_mode=DR,
                        skip_group_check=True,
                    )

            def uv_copy(ns):
                nc.vector.tensor_copy(
                    uvrow_sb[0:33, ns * S : (ns + 1) * S], srs[ns][0:33, :]
                )

            def uv_scatter():
                # scatter rows [1, 2048] -> [128, pair, sub] columns off the
                # PE: bounce through flat DRAM, where the partition-scatter
                # read pattern is expressible. The two 2048-descriptor
                # scatters go on different queues so they don't serialize;
                # results are needed only by the first srs matmul, ~10us out.
                uv_dram = dpool.tile([2, NN], F8)
                nc.sync.dma_start(uv_dram[1:2, :], uvrow_sb[32:33, :])
                nc.sync.dma_start(uv_dram[0:1, :], uvrow_sb[0:1, :])
                with nc.allow_non_contiguous_dma(
                    reason="2048-elem partition scatter, one-off"
                ):
                    nc.sync.dma_start(
                        vcol_sb[:, :, :],
                        uv_dram[1, :].rearrange("(pr i p) -> p pr i", p=P, i=2),
                    )
                    nc.gpsimd.dma_start(
                        su_sb[:, :, :, 0],
                        uv_dram[0, :].rearrange("(pr i p) -> p pr i", p=P, i=2),
                    )

            def su_fold():
                # ev = e^v, then su col0 = u*e^v, col1 = e^v. One tiny ACT
                # (between exps on the scalar queue) + two DVE ops.
                nc.scalar.activation(
                    ev_sb[:, :, :],
                    vcol_sb[:, :, :],
                    Exp,
                    bias=0.0,
                    scale=SCALE / ASCALE,
                )
                nc.vector.tensor_tensor(
                    su_sb[:, :, :, 0],
                    su_sb[:, :, :, 0],
                    ev_sb[:, :, :],
                    mybir.AluOpType.mult,
                )
                nc.vector.tensor_copy(su_sb[:, :, :, 1], ev_sb[:, :, :])

            # ---- proj: B = A R^T, DoubleRow, cast to fp8 bt ----
            def proj(qo, half, cast_on_act=False):
                t = psMain.tile([P, 2 * S], F32, tag="mm", name="mm")
                for sub in range(2):
                    ns = 2 * half + sub
                    po = t[:, sub * S : (sub + 1) * S]
                    for pr in range(2):
                        nc.tensor.matmul(
                            po[:],
                            a_sb[:, 2 * pr : 2 * pr + 2, qo * P : (qo + 1) * P],
                            rt_sb[:, ns, 2 * pr : 2 * pr + 2, :],
                            start=(pr == 0),
                            stop=(pr == 1),
                            perf_mode=DR,
                        )
                # split each cast across DVE and the (still idle) ACT engine:
                # gamma chunks 0-3 need only the first 512 columns, so the
                # DVE halves unblock the first gamma chunk ~1us after the
                # wave's last matmul
                lo = bt_sb[:, qo, half * 2 * S : half * 2 * S + S]
                hi = bt_sb[:, qo, half * 2 * S + S : (half + 1) * 2 * S]
                nc.vector.tensor_copy(lo, t[:, 0:S])
                nc.scalar.copy(hi, t[:, S : 2 * S])

            def proj_psr(qo, half, banks):
                # second-half proj waves write the (still idle) psR banks so
                # their DVE casts never gate the gamma psMain rotation
                for sub in range(2):
                    ns = 2 * half + sub
                    po = srs[banks[sub]]
                    for pr in range(2):
                        nc.tensor.matmul(
                            po[:],
                            a_sb[:, 2 * pr : 2 * pr + 2, qo * P : (qo + 1) * P],
                            rt_sb[:, ns, 2 * pr : 2 * pr + 2, :],
                            start=(pr == 0),
                            stop=(pr == 1),
                            perf_mode=DR,
                            skip_group_check=True,
                        )
                    nc.vector.tensor_copy(
                        bt_sb[:, qo, ns * S : (ns + 1) * S], po[:]
                    )

            # ---- gamma + exp ----
            ets = {}

            def gamma(mc):
                if mc % 2 == 0:
                    ets[mc // 2] = et_pool.tile(
                        [P, 2, NN], F8, tag="et", name="et"
                    )
                et = ets[mc // 2]
                for half in range(2):
                    g = psMain.tile([P, 2 * S], F32, tag="mm", name="mm")
                    for sub in range(2):
                        ns = 2 * half + sub
                        go = g[:, sub * S : (sub + 1) * S]
                        for pr in range(2):
                            nc.tensor.matmul(
                                go[:],
                                bt_sb[:, 2 * pr : 2 * pr + 2, mc * P : (mc + 1) * P],
                                rt_sb[:, ns, 2 * pr : 2 * pr + 2, :],
                                start=(pr == 0),
                                stop=(pr == 1),
                                perf_mode=DR,
                            )
                    nc.scalar.activation(
                        et[:, mc % 2, half * 2 * S : (half + 1) * 2 * S],
                        g[:],
                        Exp,
                        bias=0.0,
                        scale=SCALE / ASCALE,
                    )

            # ---- srs: s/rowsum accumulation over m chunk-pairs ----
            def srs_pair(pp):
                et = ets[pp]
                for ns in range(NS):
                    nc.tensor.matmul(
                        srs[ns][0:SUC, :],
                        su_sb[:, pp, :, :],
                        et[:, :, ns * S : (ns + 1) * S],
                        start=(pp == 0),
                        stop=(pp == NPR - 1),
                        perf_mode=DR,
                        skip_group_check=True,
                    )
                del ets[pp]

            # schedule, ordered to (a) match DMA arrival, (b) start the exp
            # stream (the critical ACT work) as early as possible, and (c)
            # spread the remaining proj/srs PE work thinly through the gamma
            # stream so ACT never starves: per gamma chunk the PE owes 1.73us
            # against ACT's 2.3us of exp, so every other chunk gets one 4-MM
            # filler (a proj wave-half or an srs pair).
            uv(0)
            uv(1)
            for qo in range(NQ):
                proj(qo, 0, cast_on_act=(qo % 2 == 1))
            gamma(0)
            uv(2)
            uv(3)
            for ns in range(NS):
                uv_copy(ns)
            uv_scatter()
            proj_psr(0, 1, (0, 1))
            gamma(1)
            proj_psr(1, 1, (2, 3))
            gamma(2)
            proj_psr(2, 1, (0, 1))
            gamma(3)
            proj_psr(3, 1, (2, 3))
            gamma(4)
            su_fold()
            # srs pair p must trail gamma(2p+3) so its exps are done when the
            # in-order PE queue reaches it
            srs_slot = {7: 0, 8: 1, 9: 2, 10: 3, 11: 4, 13: 5, 15: 6}
            for mc in range(5, 16):
                gamma(mc)
                if mc in srs_slot:
                    srs_pair(srs_slot[mc])
            srs_pair(NPR - 1)

            # copy the two result rows PSUM -> SBUF, then one DMA out.
            out_sb = big.tile([2, NN], F32, name="out_sb")
            for ns in range(NS):
                sl = slice(ns * S, (ns + 1) * S)
                # alternate DVE/ACT so the four drain copies run on two
                # engines in parallel
                if ns % 2 == 0:
                    nc.vector.tensor_copy(out_sb[:, sl], srs[ns][0:2, :])
                else:
                    nc.scalar.copy(out_sb[:, sl], srs[ns][0:2, :])
            nc.sync.dma_start(out.ap()[:], out_sb[:])
            psMain.release()
            psR.release()

    _split_multi_waits(nc)
    return nc


_NC = None


def _get_nc():
    global _NC
    if _NC is None:
        _NC = _build()
    return _NC


def kernel(R, Wq, bq, Wk, bk, Wv, bv, W1, b1, W2, b2):
    R = np.asarray(R, np.float32)
    Wq = np.asarray(Wq, np.float64)
    bq = np.asarray(bq, np.float64)
    Wk = np.asarray(Wk, np.float64)
    bk = np.asarray(bk, np.float64)
    Wv = np.asarray(Wv, np.float64)
    bv = np.asarray(bv, np.float64)
    W1 = np.asarray(W1, np.float64)
    b1 = np.asarray(b1, np.float64)
    W2 = np.asarray(W2, np.float64)
    b2 = np.asarray(b2, np.float64)

    # Collapse the linear head: winner = c.a + const, u = V c.
    c = W1.T @ W2[0]                      # [512]
    wtilde = Wv.T @ c                     # [512]
    beta = float(bv @ c)
    const = float(W2[0] @ b1 + b2[0])
    # Collapse the Q/K projections: gamma = R A R^T + v[m] (+ dropped n-term)
    at = Wk.T @ Wq                        # A^T = Wk^T Wq, [q', q]
    w2tilde = Wk.T @ bq                   # [512]
    beta2 = float(bq @ bk)

    # [p, chunk, d] with q = chunk*128 + p; x64 pre-scale for fp8 range
    a_h = np.ascontiguousarray(
        (at * ASCALE).reshape(NQ, P, DD).transpose(1, 0, 2)
    ).astype(F8NP)
    wuv_h = np.zeros((P, NQ, WUVC), F8NP)
    wuv_h[:, :, 0] = (wtilde * ASCALE).reshape(NQ, P).T.astype(F8NP)
    wuv_h[:, :, 32] = (w2tilde * ASCALE).reshape(NQ, P).T.astype(F8NP)

    in_maps = []
    for b in range(NB):
        # [p, chunk, n]: R[b].T chunked over q so each q-chunk-pair slice
        # is a regular strided DMA
        rt_h = np.ascontiguousarray(
            R[b].T.reshape(NQ, P, NS, S).transpose(1, 2, 0, 3)
        ).astype(F8NP)
        in_maps.append(
            {
                "rt": rt_h,
                "amat": a_h,
                "wuv": wuv_h,
            }
        )

    nc = _get_nc()
    res = run_bass_kernel_spmd(nc, in_maps, core_ids=list(range(N_CORES)))
    outs = np.stack([res.results[b]["out"] for b in range(NB)])   # [8,2,2048]
    # s row is 64x (raw u weights); beta rides here via s = s' + beta*rowsum
    return (
        outs[:, 0] / (np.float32(ASCALE) * outs[:, 1])
        + np.float32(beta + const)
    ).astype(np.float32)


# revision 29
# speedup vs baseline: 1.0348x; 1.0006x over previous
"""CAAN kernel for Trainium2, 8-core data-parallel (one batch row per core).

Math: the reference is
    Q = R Wq^T + bq ; K = R Wk^T + bk ; V = R Wv^T + bv
    E = exp(Q K^T / sqrt(512)) ; saat = E / rowsum(E)
    winner = (saat V) W1^T W2^T + (W2 b1 + b2)

Two algebraic collapses make most of the network disappear:

1. The W1/W2 head is linear, so with c = W1^T W2[0]:
       winner[n] = (sum_m E[n,m] u[m]) / (sum_m E[n,m]) + const,
   u = V c = R (Wv^T c) + bv.c — a per-asset scalar. The V projection and
   attention*V matmul vanish.

2. gamma = Q K^T = R A R^T + (R Wq^T bk)[n] + (R Wk^T bq)[m] + bq.bk with
   A = Wq^T Wk. The per-n term scales E rows uniformly and cancels in the
   s/rowsum ratio, so it is dropped. The per-m term v[m] rides the exp
   activation's per-partition bias slot. The Q and K projections collapse
   into a single projection B = A^T-pack @ R^T.

fp8 version: everything the PE touches is fp8-e4m3 (A and the u/v weights
pre-scaled by 64 to clear the e4m3 subnormal range). All big matmuls run
in DoubleRow perf mode (256 contraction rows per matmul) for 2x PE
throughput; the s/rowsum reduction is DoubleRow too (exp output ET is
fp8, two m-chunks per matmul). rel-err vs the f32 reference ~4.4e-3: fp8
quantization noise averages out in the softmax sums.

The critical engine end-to-end is ACT: 32 exp ops of [128,1024] (two PSUM
banks per op, amortizing the ~300ns per-op overhead) = ~37us that nothing
can shrink, so the whole schedule exists to (a) start the first exp as
early as possible and (b) never let ACT starve. Per core:
  warmup: 14 dummy matmuls un-throttle the PE HAM clock gate during the
          input-DMA ramp (cold PE runs at half clock for ~3.4us).
  uv:     8 DoubleRow matmuls -> RAW rows 64*R.wtilde / 64*R.w2tilde in
          PSUM rows 0/32. No on-device affine fixup: the 1/64 and +beta on
          u ride to the host via s = s' + beta*rowsum, v's scale folds into
          the e^v activation, and e^{SCALE*beta2} cancels in s/rowsum.
          Rows bounce through DRAM to become [128, pair, sub] fp8 columns;
          e^v then folds INTO the s/rowsum weights (su col0 = u_raw*e^v,
          col1 = e^v), keeping the v chain off the exp critical path.
  proj:   B = A R^T, DoubleRow. First n-half (m 0:1024) in psMain with
          casts split DVE/ACT per 512 so gamma(0) unblocks ~1us after the
          wave; second n-half runs as fillers through the idle psR banks so
          its DVE casts never gate the gamma psMain rotation.
  gamma:  per m-chunk: 8 DoubleRow matmuls into two [128,1024] 2-bank
          psMain tiles; exp(psum * SCALE/64) -> fp8 ET, 2-bank ACT ops.
          The two psMain tiles rotate at exactly ACT speed.
  srs:    s (row 0) and rowsum (row 1) accumulate over 8 chunk-pairs of
          DoubleRow matmuls with [u*e^v|e^v|0...] weights, spread through
          the gamma stream as PE fillers trailing their exps.
  out:    [2, 2048] f32 DMA'd out; host does
          winner = s/(64*rowsum) + beta + const.
"""

import math

import ml_dtypes
import numpy as np

import concourse.bass as bass
import concourse.mybir as mybir
import concourse.tile as tile
from concourse.bass_utils import run_bass_kernel_spmd
from concourse.vector_clock import ScopedClock


N_CORES = 8
NB, NN, DD = 8, 2048, 512  # batch, assets, feature dim
P = 128
NQ = DD // P   # q chunks (contraction)
NM = NN // P   # m chunks (key/asset rows)
NPR = NM // 2  # m chunk-pairs for the DoubleRow s/rowsum reduction
S = 512        # PSUM bank width in f32
NS = NN // S   # n slices of 512
WUVC = 48      # wuv padded col count (16-aligned for DoubleRow weight step)
SUC = 16       # su padded col count (16-aligned DoubleRow weight step)
BF16 = mybir.dt.bfloat16
F32 = mybir.dt.float32
F8 = mybir.dt.float8e4
SCALE = 1.0 / math.sqrt(float(DD))
ASCALE = 64.0  # fp8 pre-scale on A / wuv (keeps entries out of e4m3 subnormals)
BF = ml_dtypes.bfloat16
F8NP = ml_dtypes.float8_e4m3
DR = mybir.MatmulPerfMode.DoubleRow


class _TileContext(tile.TileContext):
    """Workaround for walrus rejecting >1 sem wait on the kernel-tail Drain
    ("Too many sync wait commands"): put each final wait on its own SP NoOp
    ahead of an unwaited Drain."""

    def _drain_and_barrier(self, tick_clock, wait_clock):
        nc = self.nc
        probe = nc.sync.nop(nofuse=True)
        wait_clock.add_sem_waits(
            probe.ins, ScopedClock({None: tick_clock.global_clock})
        )
        si = probe.ins.sync_info
        waits = list(si.on_wait) if si is not None else []
        if si is not None:
            si.on_wait = []
        # spread the final waits round-robin over all engines so they
        # resolve in parallel; the barrier then guarantees every wait has
        # been observed before the SP drain runs.
        engines = [nc.sync, nc.vector, nc.scalar, nc.tensor, nc.gpsimd]
        for i, w in enumerate(waits):
            n = engines[i % len(engines)].nop(nofuse=True)
            n.ins.sync_info = mybir.SyncInfo(on_wait=[w], on_update=[])
        nc.all_engine_barrier()
        nc.sync.drain()
        assert self.sems is not None
        popped = nc._tile_sem_poison_stack.pop()
        assert popped is self._sem_poison
        # clear_and_free_semaphores would range-clear every ALLOCATED sem id
        # (~200+), which walrus lowers to one op per id (~7us of tail).
        # Only ids that appear in the final instruction stream can be
        # non-zero, so hardware-clear just those; do the allocator
        # bookkeeping for the full set.
        allocated = list(self.sems.allocated().values())
        sem_nums = [
            s.num if hasattr(s, "num") else int(s) for s in allocated
        ]
        used = set()
        for fn in nc.m.functions:
            for blk in fn.blocks:
                for inst in blk.instructions:
                    si = inst.sync_info
                    if si is not None:
                        for w in si.on_wait:
                            used.add(w.id)
                        for u in si.on_update:
                            used.add(u.id)
        hw_nums = sorted(n for n in sem_nums if n in used)
        for sem_range in bass.compact_to_ranges(hw_nums):
            nc.gpsimd.dma_reset(sem_range)
            nc.gpsimd.sem_clear(sem_range)
        nc._state.prepend_free_semaphores(sem_nums)
        for poison_set in nc._tile_sem_poison_stack:
            poison_set.update(sem_nums)
        # the trailing all_engine_barrier is skipped: nothing after the
        # clear touches semaphores, and the runtime serializes executions


def _split_multi_waits(nc, maxw=1):
    """This walrus build rejects instructions carrying more than one sync
    wait ("Too many sync wait commands"). Move excess waits onto same-engine
    NoOps inserted just before the instruction: sem-ge waits are monotonic
    within the kernel, so waiting for them earlier on the same engine is
    equivalent. sem-eq waits stay on the original instruction."""
    for fn in nc.m.functions:
        for blk in fn.blocks:
            insts = blk.instructions
            if not any(
                i.sync_info is not None and len(i.sync_info.on_wait) > maxw
                for i in insts
            ):
                continue
            out = []
            for inst in insts:
                si = inst.sync_info
                if si is not None and len(si.on_wait) > maxw:
                    keep = [w for w in si.on_wait if "eq" in w.wait_mode]
                    movable = [w for w in si.on_wait if "eq" not in w.wait_mode]
                    while len(keep) < maxw and movable:
                        keep.append(movable.pop(0))
                    assert len(keep) <= maxw, (
                        f"{inst.name}: {len(keep)} non-splittable waits"
                    )
                    for w in movable:
                        nop = mybir.InstNoOp(
                            name=nc.get_next_instruction_name(), ins=[], outs=[]
                        )
                        nop.engine = inst.engine
                        nop.sync_info = mybir.SyncInfo(on_wait=[w], on_update=[])
                        out.append(nop)
                    si.on_wait = keep
                out.append(inst)
            blk.instructions = out


def _build():
    nc = bass.Bass("TRN2", target_bir_lowering=False, debug=False)

    rt = nc.dram_tensor("rt", (P, NS, NQ, S), F8, kind="ExternalInput")
    amat = nc.dram_tensor("amat", (P, NQ, DD), F8, kind="ExternalInput")
    wuv = nc.dram_tensor("wuv", (P, NQ, WUVC), F8, kind="ExternalInput")
    out = nc.dram_tensor("out", (2, NN), F32, kind="ExternalOutput")

    Ident = mybir.ActivationFunctionType.Identity
    Exp = mybir.ActivationFunctionType.Exp

    with _TileContext(nc) as tc:
        with (
            tc.tile_pool(name="const", bufs=1) as cpool,
            tc.tile_pool(name="big", bufs=1) as big,
            tc.tile_pool(name="et", bufs=6) as et_pool,
            tc.tile_pool(name="dscratch", bufs=1, space="DRAM") as dpool,
        ):
            rt_sb = cpool.tile([P, NS, NQ, S], F8, name="rt")
            a_sb = cpool.tile([P, NQ, DD], F8, name="a")
            wuv_sb = cpool.tile([P, NQ, WUVC], F8, name="wuv")
            dummy_sb = cpool.tile([P, S], BF16, name="dummy")

            # DMA order matches consumption order: wuv + the first rt
            # pair-slices feed the uv matmuls, a feeds proj wave 0.
            # rt is packed [p, n-slice, chunk, 512] so each n-slice is one
            # 256KB DMA with 2KB-contiguous partition lines (512B lines ran
            # at ~1/4 the bandwidth and starved the uv/proj ramp). `a` goes
            # first on its queue: it gates the entire proj->gamma->exp
            # pipeline, while wuv/betas feed later consumers. betas (33 tiny
            # 8B packets, slow to drain) rides behind the rt slices on sync.
            # each n-slice splits pair-wise across both queues (contiguous
            # 1KB partition lines) so slices arrive in consumption order at
            # the aggregate bandwidth of both queues
            nc.scalar.dma_start(wuv_sb[:], wuv.ap())
            nc.scalar.dma_start(a_sb[:], amat.ap())
            for ns in range(NS):
                e0, e1 = (nc.sync, nc.gpsimd) if ns % 2 == 0 else (nc.gpsimd, nc.sync)
                e0.dma_start(rt_sb[:, ns, 0:2], rt.ap()[:, ns, 0:2])
                e1.dma_start(rt_sb[:, ns, 2:4], rt.ap()[:, ns, 2:4])

            bt_sb = big.tile([P, NQ, NN], F8, name="bt")
            uvrow_sb = big.tile([33, NN], F8, name="uvrow")
            vcol_sb = big.tile([P, NPR, 2], F8, name="vcol")
            ev_sb = big.tile([P, NPR, 2], F8, name="ev")
            # su weights per m-chunk-pair: [pair, sub, col] with col 0 =
            # u*e^v, col 1 = e^v, cols 2..15 zero padding (16B DoubleRow
            # step). Folding e^v into the weights instead of biasing the exp
            # keeps the v scatter chain off the first exp's critical path:
            #   E[n,m] = e^{v[m]} E'[n,m]  =>  s = E'.(e^v u), rowsum = E'.e^v
            su_sb = big.tile([P, NPR, 2, SUC], F8, name="su")
            nc.gpsimd.memset(su_sb[:], 0.0)

            # psR: 4 banks for the s/rowsum accumulators (also reused as
            # scratch for the uv projections before srs starts).
            # psMain: 4 banks as two rotating [128,1024] 2-bank tiles so the
            # exp ACTs can cover 1024 columns per instruction.
            psR = tc.alloc_tile_pool(name="psR", bufs=1, space="PSUM")
            psMain = tc.alloc_tile_pool(name="psMain", bufs=2, space="PSUM")
            srs = [
                psR.tile([P, S], F32, tag=f"srs{ns}", name=f"srs{ns}")
                for ns in range(NS)
            ]

            # ---- PE warmup: dummy matmuls (no input deps) un-throttle the
            # HAM clock gate while the input DMAs stream in.
            nc.vector.memset(dummy_sb[:], 0.0)
            for _ in range(14):
                t = psMain.tile([P, 2 * S], F32, tag="mm", name="mm")
                nc.tensor.matmul(
                    t[:, 0:S], dummy_sb[:, 0:P], dummy_sb[:], start=True, stop=True
                )

            # ---- uv: raw u row (64*R.wtilde, row 0) and raw v row
            # (64*R.w2tilde, row 32). No on-device affine fixup at all: the
            # 1/64 and +beta on u ride to the host (s = s' + beta*rowsum),
            # and v's scale folds into the e^v activation while e^{beta2}
            # cancels in the s/rowsum ratio.
            def uv(ns):
                pur = srs[ns][0:WUVC, :]
                for pr in range(2):
                    nc.tensor.matmul(
                        pur[:],
                        wuv_sb[:, 2 * pr : 2 * pr + 2, :],
                        rt_sb[:, ns, 2 * pr : 2 * pr + 2, :],
                        start=(pr == 0),
                        stop=(pr == 1),
                        perf_mode=DR,
                        skip_group_check=True,
                    )

            def uv_copy(ns):
                nc.vector.tensor_copy(
                    uvrow_sb[0:33, ns * S : (ns + 1) * S], srs[ns][0:33, :]
                )

            def uv_scatter():
                # scatter rows [1, 2048] -> [128, pair, sub] columns off the
                # PE: bounce through flat DRAM, where the partition-scatter
                # read pattern is expressible. The two 2048-descriptor
                # scatters go on different queues so they don't serialize;
                # results are needed only by the first srs matmul, ~10us out.
                uv_dram = dpool.tile([2, NN], F8)
                nc.sync.dma_start(uv_dram[1:2, :], uvrow_sb[32:33, :])
                nc.sync.dma_start(uv_dram[0:1, :], uvrow_sb[0:1, :])
                with nc.allow_non_contiguous_dma(
                    reason="2048-elem partition scatter, one-off"
                ):
                    nc.sync.dma_start(
                        vcol_sb[:, :, :],
                        uv_dram[1, :].rearrange("(pr i p) -> p pr i", p=P, i=2),
                    )
                    nc.gpsimd.dma_start(
                        su_sb[:, :, :, 0],
                        uv_dram[0, :].rearrange("(pr i p) -> p pr i", p=P, i=2),
                    )

            def su_fold():
                # ev = e^v, then su col0 = u*e^v, col1 = e^v. One tiny ACT
                # (between exps on the scalar queue) + two DVE ops.
                nc.scalar.activation(
                    ev_sb[:, :, :],
                    vcol_sb[:, :, :],
                    Exp,
                    bias=0.0,
                    scale=SCALE / ASCALE,
                )
                nc.vector.tensor_tensor(
                    su_sb[:, :, :, 0],
                    su_sb[:, :, :, 0],
                    ev_sb[:, :, :],
                    mybir.AluOpType.mult,
                )
                nc.vector.tensor_copy(su_sb[:, :, :, 1], ev_sb[:, :, :])

            # ---- proj: B = A R^T, DoubleRow, cast to fp8 bt ----
            def proj(qo, half, cast_on_act=False):
                t = psMain.tile([P, 2 * S], F32, tag="mm", name="mm")
                for sub in range(2):
                    ns = 2 * half + sub
                    po = t[:, sub * S : (sub + 1) * S]
                    for pr in range(2):
                        nc.tensor.matmul(
                            po[:],
                            a_sb[:, 2 * pr : 2 * pr + 2, qo * P : (qo + 1) * P],
                            rt_sb[:, ns, 2 * pr : 2 * pr + 2, :],
                            start=(pr == 0),
                            stop=(pr == 1),
                            perf_mode=DR,
                        )
                # split each cast across DVE and the (still idle) ACT engine:
                # gamma chunks 0-3 need only the first 512 columns, so the
                # DVE halves unblock the first gamma chunk ~1us after the
                # wave's last matmul
                lo = bt_sb[:, qo, half * 2 * S : half * 2 * S + S]
                hi = bt_sb[:, qo, half * 2 * S + S : (half + 1) * 2 * S]
                nc.vector.tensor_copy(lo, t[:, 0:S])
                nc.scalar.copy(hi, t[:, S : 2 * S])

            def proj_psr(qo, half, banks, cast_split=False):
                # proj waves through the (still idle) psR banks so their
                # casts never gate the psMain rotation; cast_split puts the
                # second sub's cast on the (pre-exp-stream) ACT engine
                for sub in range(2):
                    ns = 2 * half + sub
                    po = srs[banks[sub]]
                    for pr in range(2):
                        nc.tensor.matmul(
                            po[:],
                            a_sb[:, 2 * pr : 2 * pr + 2, qo * P : (qo + 1) * P],
                            rt_sb[:, ns, 2 * pr : 2 * pr + 2, :],
                            start=(pr == 0),
                            stop=(pr == 1),
                            perf_mode=DR,
                            skip_group_check=True,
                        )
                    dst = bt_sb[:, qo, ns * S : (ns + 1) * S]
                    if cast_split and sub == 1:
                        nc.scalar.copy(dst, po[:])
                    else:
                        nc.vector.tensor_copy(dst, po[:])

            # ---- gamma + exp ----
            ets = {}

            def gamma(mc):
                if mc % 2 == 0:
                    ets[mc // 2] = et_pool.tile(
                        [P, 2, NN], F8, tag="et", name="et"
                    )
                et = ets[mc // 2]
                for half in range(2):
                    g = psMain.tile([P, 2 * S], F32, tag="mm", name="mm")
                    for sub in range(2):
                        ns = 2 * half + sub
                        go = g[:, sub * S : (sub + 1) * S]
                        for pr in range(2):
                            nc.tensor.matmul(
                                go[:],
                                bt_sb[:, 2 * pr : 2 * pr + 2, mc * P : (mc + 1) * P],
                                rt_sb[:, ns, 2 * pr : 2 * pr + 2, :],
                                start=(pr == 0),
                                stop=(pr == 1),
                                perf_mode=DR,
                            )
                    nc.scalar.activation(
                        et[:, mc % 2, half * 2 * S : (half + 1) * 2 * S],
                        g[:],
                        Exp,
                        bias=0.0,
                        scale=SCALE / ASCALE,
                    )

            # ---- srs: s/rowsum accumulation over m chunk-pairs ----
            def srs_pair(pp):
                et = ets[pp]
                for ns in range(NS):
                    nc.tensor.matmul(
                        srs[ns][0:SUC, :],
                        su_sb[:, pp, :, :],
                        et[:, :, ns * S : (ns + 1) * S],
                        start=(pp == 0),
                        stop=(pp == NPR - 1),
                        perf_mode=DR,
                        skip_group_check=True,
                    )
                del ets[pp]

            # schedule, ordered to (a) match DMA arrival, (b) start the exp
            # stream (the critical ACT work) as early as possible, and (c)
            # spread the remaining proj/srs PE work thinly through the gamma
            # stream so ACT never starves: per gamma chunk the PE owes 1.73us
            # against ACT's 2.3us of exp, so every other chunk gets one 4-MM
            # filler (a proj wave-half or an srs pair).
            uv(0)
            uv(1)
            # wave q2 borrows the (free until uv23) psR2/3 banks as a third
            # psum slot so np0's four waves don't serialize on their own
            # cast-driven slot recycling
            proj(0, 0)
            proj(1, 0)
            proj_psr(2, 0, (2, 3), cast_split=True)
            proj(3, 0)
            gamma(0)
            uv(2)
            uv(3)
            for ns in range(NS):
                uv_copy(ns)
            uv_scatter()
            proj_psr(0, 1, (0, 1))
            gamma(1)
            proj_psr(1, 1, (2, 3))
            gamma(2)
            proj_psr(2, 1, (0, 1))
            gamma(3)
            proj_psr(3, 1, (2, 3))
            gamma(4)
            su_fold()
            # srs pair p must trail gamma(2p+3) so its exps are done when the
            # in-order PE queue reaches it
            srs_slot = {7: 0, 8: 1, 9: 2, 10: 3, 11: 4, 13: 5, 15: 6}
            for mc in range(5, 16):
                gamma(mc)
                if mc in srs_slot:
                    srs_pair(srs_slot[mc])
            srs_pair(NPR - 1)

            # copy the two result rows PSUM -> SBUF, then one DMA out.
            out_sb = big.tile([2, NN], F32, name="out_sb")
            for ns in range(NS):
                sl = slice(ns * S, (ns + 1) * S)
                # alternate DVE/ACT so the four drain copies run on two
                # engines in parallel
                if ns % 2 == 0:
                    nc.vector.tensor_copy(out_sb[:, sl], srs[ns][0:2, :])
                else:
                    nc.scalar.copy(out_sb[:, sl], srs[ns][0:2, :])
            nc.sync.dma_start(out.ap()[:], out_sb[:])
            psMain.release()
            psR.release()

    _split_multi_waits(nc)
    return nc


_NC = None


def _get_nc():
    global _NC
    if _NC is None:
        _NC = _build()
    return _NC


def kernel(R, Wq, bq, Wk, bk, Wv, bv, W1, b1, W2, b2):
    R = np.asarray(R, np.float32)
    Wq = np.asarray(Wq, np.float64)
    bq = np.asarray(bq, np.float64)
    Wk = np.asarray(Wk, np.float64)
    bk = np.asarray(bk, np.float64)
    Wv = np.asarray(Wv, np.float64)
    bv = np.asarray(bv, np.float64)
    W1 = np.asarray(W1, np.float64)
    b1 = np.asarray(b1, np.float64)
    W2 = np.asarray(W2, np.float64)
    b2 = np.asarray(b2, np.float64)

    # Collapse the linear head: winner = c.a + const, u = V c.
    c = W1.T @ W2[0]                      # [512]
    wtilde = Wv.T @ c                     # [512]
    beta = float(bv @ c)
    const = float(W2[0] @ b1 + b2[0])
    # Collapse the Q/K projections: gamma = R A R^T + v[m] (+ dropped n-term)
    at = Wk.T @ Wq                        # A^T = Wk^T Wq, [q', q]
    w2tilde = Wk.T @ bq                   # [512]
    beta2 = float(bq @ bk)

    # [p, chunk, d] with q = chunk*128 + p; x64 pre-scale for fp8 range
    a_h = np.ascontiguousarray(
        (at * ASCALE).reshape(NQ, P, DD).transpose(1, 0, 2)
    ).astype(F8NP)
    wuv_h = np.zeros((P, NQ, WUVC), F8NP)
    wuv_h[:, :, 0] = (wtilde * ASCALE).reshape(NQ, P).T.astype(F8NP)
    wuv_h[:, :, 32] = (w2tilde * ASCALE).reshape(NQ, P).T.astype(F8NP)

    in_maps = []
    for b in range(NB):
        # [p, chunk, n]: R[b].T chunked over q so each q-chunk-pair slice
        # is a regular strided DMA
        rt_h = np.ascontiguousarray(
            R[b].T.reshape(NQ, P, NS, S).transpose(1, 2, 0, 3)
        ).astype(F8NP)
        in_maps.append(
            {
                "rt": rt_h,
                "amat": a_h,
                "wuv": wuv_h,
            }
        )

    nc = _get_nc()
    res = run_bass_kernel_spmd(nc, in_maps, core_ids=list(range(N_CORES)))
    outs = np.stack([res.results[b]["out"] for b in range(NB)])   # [8,2,2048]
    # s row is 64x (raw u weights); beta rides here via s = s' + beta*rowsum
    return (
        outs[:, 0] / (np.float32(ASCALE) * outs[:, 1])
        + np.float32(beta + const)
    ).astype(np.float32)
